# revision 17
# baseline (speedup 1.0000x reference)
"""BiGraphContrastLayer (GAT + drop-edge contrast) on 8 TRN2 NeuronCores.

Strategy: dst-node partitioning (2500 dst nodes per core, no collectives).
 - Phase P (per core, replicated src work): z = feat @ fc_w, el/er attention
   logits; src rows packed into a bf16 DRAM gather table [z(128) | el(8)].
   feat arrives host-pretransposed so no on-device transposes are needed;
   one fused matmul per 128-node tile computes z and el|er together.
 - Phase E: per 128-dst window, dma_gather z rows by edge_src (4 SWDGE
   queues round-robin: descriptor processing is the gather bottleneck and
   parallelizes across queues), batched one-hot build, per-tile PE
   transposes for the er-expansion matmuls, segment-softmax (shift m=0 is
   exact: logits are O(1)) and weighted segment-sums via TensorE matmuls
   accumulating in PSUM. The negative graph (1% of edges dropped) is
   aggregated as pos - dropped, so gathers are shared between graphs.
 - Finalize: self-loop term, normalize (x8 folded in for the head-mean),
   PReLU, head-mean, then a batched cosine/LSE tail. Host takes log of the
   summed per-core partials and concatenates h_pos shards.
"""

import sys
import numpy as np

sys.path.insert(0, "/opt/trn_rl_repo")

import antenv  # noqa: E402

if "/opt/trn_rl_repo/antenv" not in antenv.__path__:
    antenv.__path__.append("/opt/trn_rl_repo/antenv")

import concourse.bass as bass  # noqa: E402
import concourse.mybir as mybir  # noqa: E402
from concourse import library_config  # noqa: E402
from concourse import library_overlay  # noqa: E402
from concourse.tile import TileContext  # noqa: E402
from concourse.bass_utils import run_bass_kernel_spmd  # noqa: E402

dt = mybir.dt
AT = mybir.AluOpType
AF = mybir.ActivationFunctionType
AX = mybir.AxisListType

CFG = dict(
    N_SRC=20000,
    N_DST=20000,
    E=320000,
    H=8,
    D=16,
    IN=128,
    NCORES=8,
    TEM=0.7,
    SLOPE=0.2,
)
ELEM = 256  # bf16 elems per table row (512B): z[0:128], el[128:136], pad
NQ = 4  # SWDGE queues for gathers

MAX_SYNC_WAITS = 1


def _split_sync_waits(nc, maxw=MAX_SYNC_WAITS):
    """walrus here rejects >~2 sync waits per instruction; split extras onto
    InstNoOp carriers inserted before, same engine (stream order is kept)."""
    for _name, handle in nc.bb_map.items():
        bb = handle.bb
        insts = bb.instructions
        i = 0
        while i < len(insts):
            ins = insts[i]
            si = ins.sync_info
            if si is not None and si.on_wait and len(si.on_wait) > maxw:
                waits = list(si.on_wait)
                si.on_wait = waits[:maxw]
                extra = waits[maxw:]
                carriers = []
                for k in range(0, len(extra), maxw):
                    nop = mybir.InstNoOp(
                        name=f"{ins.name}-sw{k}",
                        engine=ins.engine,
                        bass_nofuse=True,
                        sync_info=mybir.SyncInfo(
                            on_wait=extra[k : k + maxw], on_update=[]
                        ),
                    )
                    carriers.append(nop)
                for j, nop in enumerate(carriers):
                    nc.register_instruction(nop, overwrite=True)
                    insts.insert(i + j, nop)
                i += len(carriers)
            i += 1


def _view(tile_ap, offset_ap, dims):
    """AP with explicit free dims; partition dim taken from tile_ap."""
    return bass.AP(offset_ap.tensor, offset_ap.offset, [tile_ap.ap[0]] + dims)


def build_graph(T, TD, has_bias, cfg=CFG):
    H, D, IN = cfg["H"], cfg["D"], cfg["IN"]
    HD = H * D
    DPC = cfg["N_DST"] // cfg["NCORES"]
    NW = (DPC + 127) // 128
    NSP = ((cfg["N_SRC"] + 127) // 128) * 128
    NSRC_T = NSP // 128
    inv_tem = 1.0 / cfg["TEM"]
    slope = cfg["SLOPE"]
    TT = T + TD

    nc = bass.Bass(num_swdge_queues=NQ, dynamic_dma_scratch_size=49152)
    f32, bf16, i16 = dt.float32, dt.bfloat16, dt.int16

    featT_src = nc.dram_tensor("featT_src", [IN, NSP], f32, kind="ExternalInput")
    featT_dst = nc.dram_tensor("featT_dst", [IN, NW * 128], f32, kind="ExternalInput")
    fc_w = nc.dram_tensor("fc_w", [IN, HD], f32, kind="ExternalInput")
    attn_l = nc.dram_tensor("attn_l", [H, D], f32, kind="ExternalInput")
    attn_r = nc.dram_tensor("attn_r", [H, D], f32, kind="ExternalInput")
    bias_in = nc.dram_tensor("bias", [1, HD], f32, kind="ExternalInput")
    prelu_in = nc.dram_tensor("prelu", [1, 1], f32, kind="ExternalInput")
    gm_idx = nc.dram_tensor("gm_idx", [128, NW * T * 8], i16, kind="ExternalInput")
    gd_idx = nc.dram_tensor("gd_idx", [128, NW * TD * 8], i16, kind="ExternalInput")
    mask_in = nc.dram_tensor("mask", [128, NW], f32, kind="ExternalInput")
    ident_in = nc.dram_tensor("ident_bf", [128, 128], bf16, kind="ExternalInput")
    oh_in = nc.dram_tensor("oh_all", [128, NW * TT * 128], bf16, kind="ExternalInput")
    ohT_in = nc.dram_tensor("ohT_all", [128, NW * TT * 128], bf16, kind="ExternalInput")

    hpos = nc.dram_tensor("hpos", [NW * 128, D], f32, kind="ExternalOutput")
    scal = nc.dram_tensor("scal", [1, 1], f32, kind="ExternalOutput")

    ztable = nc.dram_tensor("ztable", [NSP, ELEM], bf16)

    nc.gpsimd.load_library(library_config.mlp)

    with TileContext(nc) as tc:
        import contextlib

        ctx = contextlib.ExitStack()
        with ctx:
            cp = ctx.enter_context(tc.tile_pool(name="const", bufs=1))

            ident_bf = cp.tile([128, 128], bf16)
            nc.sync.dma_start(ident_bf[:], ident_in[:])
            gm_sb = cp.tile([128, NW * T * 8], i16)
            nc.sync.dma_start(gm_sb[:], gm_idx[:])
            gd_sb = cp.tile([128, NW * TD * 8], i16)
            nc.sync.dma_start(gd_sb[:], gd_idx[:])
            mask_sb = cp.tile([128, NW], f32)
            nc.sync.dma_start(mask_sb[:], mask_in[:])

            # fcwq = [fc_w | fc_w @ (WL|WR)] : one rhs for the fused phase-P matmul
            wlr = cp.tile([128, 2 * H], f32)
            nc.vector.memset(wlr[:], 0.0)
            for h in range(H):
                nc.sync.dma_start(
                    wlr[h * D : (h + 1) * D, h : h + 1],
                    bass.AP(attn_l, h * D, [[1, D], [1, 1]]),
                )
                nc.sync.dma_start(
                    wlr[h * D : (h + 1) * D, H + h : H + h + 1],
                    bass.AP(attn_r, h * D, [[1, D], [1, 1]]),
                )
            ones_row = cp.tile([1, 128], f32)
            nc.vector.memset(ones_row[:], 1.0)
            ones_col = cp.tile([128, 1], f32)
            nc.vector.memset(ones_col[:], 1.0)
            prelu_sb = cp.tile([1, 1], f32)
            nc.sync.dma_start(prelu_sb[:], prelu_in[:])

            fcwq = cp.tile([128, HD + 2 * H], f32)
            nc.sync.dma_start(fcwq[:, 0:HD], fc_w[:])
            bias_rep = cp.tile([128, HD], f32)
            prelu_bc = cp.tile([128, 1], f32)
            with tc.tile_pool(name="initps", bufs=1, space="PSUM") as ips:
                identf = cp.tile([128, 128], f32)
                nc.vector.tensor_copy(identf[:], ident_bf[:])
                fcwT_ps = ips.tile([128, HD], f32, tag="t")
                nc.tensor.transpose(fcwT_ps[:], fcwq[:, 0:HD], identf[:])
                fcwT = cp.tile([128, HD], f32)
                nc.vector.tensor_copy(fcwT[:], fcwT_ps[:])
                q_ps = ips.tile([128, 2 * H], f32, tag="t2")
                nc.tensor.matmul(q_ps[:], fcwT[:], wlr[:])
                nc.vector.tensor_copy(fcwq[:, HD : HD + 2 * H], q_ps[:])
                if has_bias:
                    bias_row = cp.tile([1, HD], f32)
                    nc.sync.dma_start(bias_row[:], bias_in[:])
                    br_ps = ips.tile([128, HD], f32, tag="t3")
                    nc.tensor.matmul(br_ps[:], ones_row[:], bias_row[:])
                    nc.vector.tensor_copy(bias_rep[:], br_ps[:])
                pr_ps = ips.tile([128, 1], f32, tag="t4")
                nc.tensor.matmul(pr_ps[:], ones_row[:], prelu_sb[:])
                nc.vector.tensor_copy(prelu_bc[:], pr_ps[:])

            zdst_pool = ctx.enter_context(tc.tile_pool(name="zdst", bufs=NW))
            elr_pool = ctx.enter_context(tc.tile_pool(name="elr", bufs=NW))
            erbf_pool = ctx.enter_context(tc.tile_pool(name="erbf", bufs=NW))
            zdst_t, elr_t, erbf_t = [], [], []

            # ---- Phase P ----
            PB = 3  # node tiles per psum batch (3*144 <= 512 f32 psum bank)
            LB = 12  # node tiles per input DMA
            HE = HD + 2 * H
            with (
                tc.tile_pool(name="pp", bufs=4) as pp,
                tc.tile_pool(name="pps", bufs=3, space="PSUM") as pps,
            ):
                def node_tile(src_ap):
                    ft = pp.tile([128, 128], f32, tag="ft")
                    nc.sync.dma_start(ft[:], src_ap)
                    ze_ps = pps.tile([128, HD + 2 * H], f32, tag="ze")
                    nc.tensor.matmul(ze_ps[:], ft[:], fcwq[:])
                    return ze_ps

                for l0 in range(0, NSRC_T, LB):
                    lb = min(LB, NSRC_T - l0)
                    ftb = pp.tile([128, LB * 128], f32, tag="ftb")
                    nc.sync.dma_start(
                        ftb[:, 0 : lb * 128],
                        featT_src[:, l0 * 128 : (l0 + lb) * 128],
                    )
                    for b0 in range(0, lb, PB):
                        pb = min(PB, lb - b0)
                        zeb = pps.tile([128, PB, HE], f32, tag="zeb")
                        for j in range(pb):
                            nc.tensor.matmul(
                                zeb[:, j, :],
                                ftb[:, (b0 + j) * 128 : (b0 + j + 1) * 128],
                                fcwq[:],
                            )
                        zt = pp.tile([128, PB, HD + H], bf16, tag="zt")
                        ze_v = _view(
                            zeb[:], zeb[:, 0, 0 : HD + H], [[HE, pb], [1, HD + H]]
                        )
                        nc.vector.tensor_copy(zt[:, 0:pb, :], ze_v)
                        dst = bass.AP(
                            ztable, (l0 + b0) * 128 * ELEM,
                            [[ELEM, 128], [128 * ELEM, pb], [1, HD + H]],
                        )
                        nc.scalar.dma_start(dst, zt[:, 0:pb, :])

                for w in range(NW):
                    ze_ps = node_tile(featT_dst[:, w * 128 : (w + 1) * 128])
                    zd = zdst_pool.tile([128, HD], f32)
                    nc.vector.tensor_copy(zd[:], ze_ps[:, 0:HD])
                    el = elr_pool.tile([128, 2 * H], f32)
                    nc.vector.tensor_copy(el[:], ze_ps[:, HD : HD + 2 * H])
                    eb = erbf_pool.tile([128, H], bf16)
                    nc.vector.tensor_copy(eb[:], ze_ps[:, HD + H : HD + 2 * H])
                    zdst_t.append(zd)
                    elr_t.append(el)
                    erbf_t.append(eb)

            # ---- Phase E ----
            ep = ctx.enter_context(tc.tile_pool(name="ep", bufs=3))
            gpool = ctx.enter_context(tc.tile_pool(name="gp", bufs=6))
            finp = ctx.enter_context(tc.tile_pool(name="finp", bufs=3))
            ps_er = ctx.enter_context(tc.tile_pool(name="psE", bufs=2, space="PSUM"))
            ps_sm = ctx.enter_context(tc.tile_pool(name="psM", bufs=2, space="PSUM"))
            ps_sd = ctx.enter_context(tc.tile_pool(name="psD", bufs=2, space="PSUM"))

            nm_buf = cp.tile([128, NW], f32)
            na_buf = cp.tile([128, NW], f32)
            nb_buf = cp.tile([128, NW], f32)

            st = {}

            def stage_gather(w):
                gm = gpool.tile([128, T, ELEM], bf16, tag="gm")
                nc.gpsimd.dma_gather(
                    gm[:], ztable[:], gm_sb[:, w * T * 8 : (w + 1) * T * 8],
                    T * 128, T * 128, ELEM, single_packet=False,
                    queue_num=w % NQ,
                )
                gd = gpool.tile([128, TD, ELEM], bf16, tag="gd")
                nc.gpsimd.dma_gather(
                    gd[:], ztable[:], gd_sb[:, w * TD * 8 : (w + 1) * TD * 8],
                    TD * 128, TD * 128, ELEM, single_packet=False,
                    queue_num=(w + 2) % NQ,
                )
                oh = gpool.tile([128, TT, 128], bf16, tag="oh")
                nc.sync.dma_start(
                    oh[:], oh_in[:, w * TT * 128 : (w + 1) * TT * 128]
                )
                ohT = gpool.tile([128, TT * 128], bf16, tag="ohTs")
                nc.scalar.dma_start(
                    ohT[:], ohT_in[:, w * TT * 128 : (w + 1) * TT * 128]
                )
                st[w] = dict(gm=gm, gd=gd, oh=oh, ohT=ohT)

            def stage_compute(w):
                s = st[w]
                gm, gd, oh, ohT = s["gm"], s["gd"], s["oh"], s["ohT"]
                er_ps = ps_er.tile([128, TT * H], f32, tag="er")
                for t in range(TT):
                    nc.tensor.matmul(
                        er_ps[:, t * H : (t + 1) * H],
                        ohT[:, t * 128 : (t + 1) * 128],
                        erbf_t[w][:],
                    )
                ew = ep.tile([128, TT * H], f32, tag="ew")
                el_m = _view(gm[:], gm[:, 0, HD : HD + H], [[ELEM, T], [1, H]])
                nc.vector.scalar_tensor_tensor(
                    ew[:, 0 : T * H], er_ps[:, 0 : T * H], 1.0, el_m, AT.mult, AT.add
                )
                el_d = _view(gd[:], gd[:, 0, HD : HD + H], [[ELEM, TD], [1, H]])
                nc.vector.scalar_tensor_tensor(
                    ew[:, T * H : TT * H], er_ps[:, T * H : TT * H], 1.0, el_d,
                    AT.mult, AT.add,
                )
                lk = ep.tile([128, TT * H], f32, tag="lk")
                nc.vector.scalar_tensor_tensor(
                    lk[:], ew[:], slope, ew[:], AT.mult, AT.max
                )
                alr = ep.tile([128, TT * H], bf16, tag="alr")
                nc.scalar.activation(alr[:], lk[:], AF.Exp)
                al_exp = ep.tile([128, TT, HD], bf16, tag="alx")
                lk_v = _view(lk[:], lk[:], [[1, TT * H], [0, D]])
                nc.scalar.activation(al_exp[:], lk_v, AF.Exp)
                msz = ep.tile([128, TT, HD], bf16, tag="msz")
                gm_v = _view(gm[:], gm[:, 0, 0:HD], [[ELEM, T], [1, HD]])
                nc.vector.tensor_tensor(msz[:, 0:T, :], gm_v, al_exp[:, 0:T, :], AT.mult)
                gd_v = _view(gd[:], gd[:, 0, 0:HD], [[ELEM, TD], [1, HD]])
                nc.vector.tensor_tensor(msz[:, T:TT, :], gd_v, al_exp[:, T:TT, :], AT.mult)
                seg_m = ps_sm.tile([128, HD + H], f32, tag="sm")
                seg_d = ps_sd.tile([128, HD + H], f32, tag="sd")
                for t in range(TT):
                    tgt = seg_m if t < T else seg_d
                    first = t == 0 or t == T
                    last = t == T - 1 or t == TT - 1
                    nc.tensor.matmul(
                        tgt[:, 0:HD], oh[:, t, :], msz[:, t, :],
                        start=first, stop=last,
                    )
                for t in range(TT):
                    tgt = seg_m if t < T else seg_d
                    first = t == 0 or t == T
                    last = t == T - 1 or t == TT - 1
                    nc.tensor.matmul(
                        tgt[:, HD : HD + H], oh[:, t, :],
                        alr[:, t * H : (t + 1) * H],
                        start=first, stop=last,
                    )
                s["seg_m"] = seg_m
                s["seg_d"] = seg_d

            def stage_final(w):
                s = st.pop(w)
                seg_m, seg_d = s["seg_m"], s["seg_d"]
                es = finp.tile([128, H], f32, tag="es")
                nc.vector.tensor_tensor(
                    es[:], elr_t[w][:, 0:H], elr_t[w][:, H : 2 * H], AT.add
                )
                nc.vector.scalar_tensor_tensor(es[:], es[:], slope, es[:], AT.mult, AT.max)
                ws_ = finp.tile([128, H], f32, tag="ws")
                nc.scalar.activation(ws_[:], es[:], AF.Exp)

                dp = finp.tile([128, H], f32, tag="dp")
                nc.vector.tensor_tensor(dp[:], seg_m[:, HD : HD + H], ws_[:], AT.add)
                dn = finp.tile([128, H], f32, tag="dn")
                nc.vector.tensor_tensor(dn[:], dp[:], seg_d[:, HD : HD + H], AT.subtract)
                nc.vector.tensor_scalar(dp[:], dp[:], float(H), None, AT.mult)
                nc.vector.tensor_scalar(dn[:], dn[:], float(H), None, AT.mult)
                rp = finp.tile([128, H], f32, tag="rp")
                nc.vector.reciprocal(rp[:], dp[:])
                rn = finp.tile([128, H], f32, tag="rn")
                nc.vector.reciprocal(rn[:], dn[:])

                ws_e = finp.tile([128, HD], f32, tag="wse")
                ws_v = _view(ws_[:], ws_[:, 0:H], [[1, H], [0, D]])
                nc.scalar.copy(ws_e[:], ws_v)
                rp_e = finp.tile([128, HD], f32, tag="rpe")
                rp_v = _view(rp[:], rp[:, 0:H], [[1, H], [0, D]])
                nc.scalar.copy(rp_e[:], rp_v)
                rn_e = finp.tile([128, HD], f32, tag="rne")
                rn_v = _view(rn[:], rn[:, 0:H], [[1, H], [0, D]])
                nc.scalar.copy(rn_e[:], rn_v)
                sm_ = finp.tile([128, HD], f32, tag="smv")
                nc.vector.tensor_tensor(sm_[:], zdst_t[w][:], ws_e[:], AT.mult)
                rstp_r = finp.tile([128, HD], f32, tag="rpr")
                nc.vector.tensor_tensor(rstp_r[:], seg_m[:, 0:HD], sm_[:], AT.add)
                rstn_r = finp.tile([128, HD], f32, tag="rnr")
                nc.vector.tensor_tensor(rstn_r[:], rstp_r[:], seg_d[:, 0:HD], AT.subtract)

                rstp = finp.tile([128, HD], f32, tag="rstp")
                nc.vector.tensor_tensor(rstp[:], rstp_r[:], rp_e[:], AT.mult)
                rstn = finp.tile([128, HD], f32, tag="rstn")
                nc.vector.tensor_tensor(rstn[:], rstn_r[:], rn_e[:], AT.mult)

                if has_bias:
                    nc.vector.tensor_tensor(rstp[:], rstp[:], bias_rep[:], AT.add)
                    nc.vector.tensor_tensor(rstn[:], rstn[:], bias_rep[:], AT.add)
                nc.vector.scalar_tensor_tensor(
                    rstp[:], rstp[:], prelu_bc[:, 0:1], rstp[:], AT.mult, AT.max
                )
                nc.vector.scalar_tensor_tensor(
                    rstn[:], rstn[:], prelu_bc[:, 0:1], rstn[:], AT.mult, AT.max
                )

                h8p = finp.tile([128, D], f32, tag="h8p")
                nc.vector.tensor_reduce(
                    h8p[:], _view(rstp[:], rstp[:, 0:HD], [[1, D], [D, H]]),
                    AX.X, AT.add,
                )
                h8n = finp.tile([128, D], f32, tag="h8n")
                nc.vector.tensor_reduce(
                    h8n[:], _view(rstn[:], rstn[:, 0:HD], [[1, D], [D, H]]),
                    AX.X, AT.add,
                )
                nc.scalar.dma_start(hpos[w * 128 : (w + 1) * 128, :], h8p[:])

                scr = finp.tile([128, D], f32, tag="scr")
                nc.vector.tensor_tensor(scr[:], h8p[:], h8n[:], AT.mult)
                nc.vector.tensor_reduce(nm_buf[:, w : w + 1], scr[:], AX.X, AT.add)
                nc.vector.tensor_tensor(scr[:], h8p[:], h8p[:], AT.mult)
                nc.vector.tensor_reduce(na_buf[:, w : w + 1], scr[:], AX.X, AT.add)
                nc.vector.tensor_tensor(scr[:], h8n[:], h8n[:], AT.mult)
                nc.vector.tensor_reduce(nb_buf[:, w : w + 1], scr[:], AX.X, AT.add)

            # skewed software pipeline: gather(w) || compute(w-1) || final(w-2)
            for w in range(NW + 2):
                if w < NW:
                    stage_gather(w)
                if 1 <= w < NW + 1:
                    stage_compute(w - 1)
                if w >= 2:
                    stage_final(w - 2)

            # ---- batched cosine / LSE tail ----
            pr2 = cp.tile([128, NW], f32)
            nc.vector.tensor_tensor(pr2[:], na_buf[:], nb_buf[:], AT.mult)
            nc.vector.tensor_scalar(pr2[:], pr2[:], 1e-30, None, AT.max)
            sq = cp.tile([128, NW], f32)
            nc.scalar.activation(sq[:], pr2[:], AF.Sqrt)
            rr = cp.tile([128, NW], f32)
            nc.vector.reciprocal(rr[:], sq[:])
            cosv = cp.tile([128, NW], f32)
            nc.vector.tensor_tensor(cosv[:], nm_buf[:], rr[:], AT.mult)
            ctr = cp.tile([128, NW], f32)
            nc.scalar.activation(ctr[:], cosv[:], AF.Exp, scale=inv_tem)
            nc.vector.tensor_tensor(ctr[:], ctr[:], mask_sb[:], AT.mult)
            acc = cp.tile([128, 1], f32)
            nc.vector.tensor_reduce(acc[:], ctr[:], AX.X, AT.add)
            with tc.tile_pool(name="fps", bufs=1, space="PSUM") as fps:
                tot_ps = fps.tile([1, 1], f32)
                nc.tensor.matmul(tot_ps[:], acc[:], ones_col[:])
                tot = cp.tile([1, 1], f32)
                nc.vector.tensor_copy(tot[:], tot_ps[:])
                nc.sync.dma_start(scal[:], tot[:])

    library_overlay.lower_extended_insts(nc)
    _split_sync_waits(nc)
    return nc


# ---------------------------------------------------------------------------
# host side
# ---------------------------------------------------------------------------

_GRAPH_CACHE = {}


def prep_inputs(feat, fc_w, attn_l, attn_r, bias, prelu_a, edge_src, edge_dst,
                neg_idx, cfg=CFG):
    NS, ND, E = cfg["N_SRC"], cfg["N_DST"], cfg["E"]
    NC_, H, D, IN = cfg["NCORES"], cfg["H"], cfg["D"], cfg["IN"]
    DPC = ND // NC_
    NW = (DPC + 127) // 128
    NSP = ((NS + 127) // 128) * 128

    feat = np.asarray(feat, np.float32)
    edge_src = np.asarray(edge_src, np.int64)
    edge_dst = np.asarray(edge_dst, np.int64)
    neg_idx = np.asarray(neg_idx, np.int64)

    keep = np.zeros(E, bool)
    keep[neg_idx] = True

    order = np.argsort(edge_dst, kind="stable")
    src_s = edge_src[order]
    dst_s = edge_dst[order]
    drop_s = ~keep[order]

    ld = dst_s - NS
    core = ld // DPC
    lw = (ld % DPC) // 128
    rel = (ld % DPC) % 128

    key = core * NW + lw
    main_counts = np.bincount(key, minlength=NC_ * NW)
    drop_counts = np.bincount(key[drop_s], minlength=NC_ * NW)
    T = max(1, int(-(-main_counts.max() // 128)))
    TD = max(1, int(-(-drop_counts.max() // 128)))

    def build_imgs(sel_src, sel_key, sel_rel, TL):
        o2 = np.argsort(sel_key, kind="stable")
        ss, kk, rr = sel_src[o2], sel_key[o2], sel_rel[o2]
        starts = np.searchsorted(kk, np.arange(NC_ * NW))
        ends = np.searchsorted(kk, np.arange(NC_ * NW) + 1)
        idx_imgs = np.zeros((NC_, 128, NW * TL * 8), np.int16)
        rel_imgs = np.full((NC_, 128, NW * TL), -1.0, np.float32)
        npad = TL * 128
        for c in range(NC_):
            for w in range(NW):
                k = c * NW + w
                s, e = starts[k], ends[k]
                n = e - s
                idx = np.zeros(npad, np.int16)
                idx[:n] = ss[s:e].astype(np.int16)
                rl = np.full(npad, -1.0, np.float32)
                rl[:n] = rr[s:e].astype(np.float32)
                img16 = idx.reshape(TL * 8, 16).T
                idx_imgs[c][:, w * TL * 8 : (w + 1) * TL * 8] = np.tile(img16, (8, 1))
                rel_imgs[c][:, w * TL : (w + 1) * TL] = rl.reshape(TL, 128).T
        return idx_imgs, rel_imgs

    gm_imgs, relm_imgs = build_imgs(src_s, key, rel, T)
    gd_imgs, reld_imgs = build_imgs(src_s[drop_s], key[drop_s], rel[drop_s], TD)
    TT = T + TD

    featT_src = np.zeros((IN, NSP), np.float32)
    featT_src[:, :NS] = feat[:NS].T

    p = np.arange(128)[:, None]
    k = np.arange(NW)[None, :]
    mask = ((k * 128 + p) < DPC).astype(np.float32)

    try:
        import ml_dtypes
        bfdt = ml_dtypes.bfloat16
    except ImportError:
        import jax.numpy as jnp
        bfdt = jnp.bfloat16
    ident_bf = np.eye(128, dtype=np.float32).astype(bfdt)
    ar128 = np.arange(128, dtype=np.float32)

    def build_onehots(relm_c, reld_c):
        # rel images: [128(e), NW*TL]; returns oh/ohT [128, NW*TT*128] bf16
        rm = relm_c.reshape(128, NW, T)
        rd = reld_c.reshape(128, NW, TD)
        rel_all = np.concatenate([rm, rd], axis=2)  # [128, NW, TT]
        oh = (rel_all[:, :, :, None] == ar128[None, None, None, :])
        ohb = oh.astype(bfdt).reshape(128, NW * TT * 128)
        # ohT[d, w, t, e] = (rel_all[e, w, t] == d)
        ohT = (rel_all.transpose(1, 2, 0)[None, :, :, :] ==
               ar128[:, None, None, None])
        ohTb = ohT.astype(bfdt).reshape(128, NW * TT * 128)
        return ohb, ohTb

    has_bias = bool(np.any(np.asarray(bias)))

    in_maps = []
    for c in range(NC_):
        oh_c, ohT_c = build_onehots(relm_imgs[c], reld_imgs[c])
        fdT = np.zeros((IN, NW * 128), np.float32)
        fdT[:, :DPC] = feat[NS + c * DPC : NS + (c + 1) * DPC].T
        in_maps.append(
            dict(
                featT_src=featT_src,
                featT_dst=fdT,
                fc_w=np.asarray(fc_w, np.float32),
                attn_l=np.asarray(attn_l, np.float32),
                attn_r=np.asarray(attn_r, np.float32),
                bias=np.asarray(bias, np.float32).reshape(1, -1),
                prelu=np.asarray(prelu_a, np.float32).reshape(1, 1),
                gm_idx=gm_imgs[c],
                gd_idx=gd_imgs[c],
                mask=mask,
                ident_bf=ident_bf,
                oh_all=oh_c,
                ohT_all=ohT_c,
            )
        )
    return in_maps, T, TD, has_bias


def run(inputs, trace=False, cfg=CFG):
    in_maps, T, TD, has_bias = prep_inputs(**inputs, cfg=cfg)
    ck = (T, TD, has_bias, id(cfg) if cfg is not CFG else 0)
    if ck not in _GRAPH_CACHE:
        _GRAPH_CACHE[ck] = build_graph(T, TD, has_bias, cfg)
    nc = _GRAPH_CACHE[ck]
    if trace:
        import antenv.axon_hooks as ah
        ah.register_default()
    res = run_bass_kernel_spmd(
        nc, in_maps, core_ids=list(range(cfg["NCORES"])), trace=trace
    )
    NC_, ND = cfg["NCORES"], cfg["N_DST"]
    DPC = ND // NC_
    hp = np.concatenate([res.results[c]["hpos"][:DPC] for c in range(NC_)], axis=0)
    s = np.sum([np.float64(res.results[c]["scal"][0, 0]) for c in range(NC_)])
    loss = np.float32(np.log(s))
    return (np.asarray(loss, np.float32), hp), res


def kernel(**inputs):
    out, _res = run(inputs, trace=False)
    return out


# revision 18
# speedup vs baseline: 1.1763x; 1.1763x over previous
"""BiGraphContrastLayer (GAT + drop-edge contrast) on 8 TRN2 NeuronCores.

Strategy: dst-node partitioning (2500 dst nodes per core, no collectives).
 - Phase P (per core, replicated src work): z = feat @ fc_w, el/er attention
   logits; src rows packed into a bf16 DRAM gather table [z(128) | el(8)].
   feat arrives host-pretransposed so no on-device transposes are needed;
   one fused matmul per 128-node tile computes z and el|er together.
 - Phase E: per 128-dst window, dma_gather z rows by edge_src (4 SWDGE
   queues round-robin: descriptor processing is the gather bottleneck and
   parallelizes across queues), batched one-hot build, per-tile PE
   transposes for the er-expansion matmuls, segment-softmax (shift m=0 is
   exact: logits are O(1)) and weighted segment-sums via TensorE matmuls
   accumulating in PSUM. The negative graph (1% of edges dropped) is
   aggregated as pos - dropped, so gathers are shared between graphs.
 - Finalize: self-loop term, normalize (x8 folded in for the head-mean),
   PReLU, head-mean, then a batched cosine/LSE tail. Host takes log of the
   summed per-core partials and concatenates h_pos shards.
"""

import sys
import numpy as np

sys.path.insert(0, "/opt/trn_rl_repo")

import antenv  # noqa: E402

if "/opt/trn_rl_repo/antenv" not in antenv.__path__:
    antenv.__path__.append("/opt/trn_rl_repo/antenv")

import concourse.bass as bass  # noqa: E402
import concourse.mybir as mybir  # noqa: E402
from concourse import library_config  # noqa: E402
from concourse import library_overlay  # noqa: E402
from concourse.tile import TileContext  # noqa: E402
from concourse.bass_utils import run_bass_kernel_spmd  # noqa: E402

dt = mybir.dt
AT = mybir.AluOpType
AF = mybir.ActivationFunctionType
AX = mybir.AxisListType

CFG = dict(
    N_SRC=20000,
    N_DST=20000,
    E=320000,
    H=8,
    D=16,
    IN=128,
    NCORES=8,
    TEM=0.7,
    SLOPE=0.2,
)
ELEM = 256  # bf16 elems per table row (512B): z[0:128], el[128:136], pad
NQ = 4  # SWDGE queues for gathers

MAX_SYNC_WAITS = 1


def _split_sync_waits(nc, maxw=MAX_SYNC_WAITS):
    """walrus here rejects >~2 sync waits per instruction; split extras onto
    InstNoOp carriers inserted before, same engine (stream order is kept)."""
    for _name, handle in nc.bb_map.items():
        bb = handle.bb
        insts = bb.instructions
        i = 0
        while i < len(insts):
            ins = insts[i]
            si = ins.sync_info
            if si is not None and si.on_wait and len(si.on_wait) > maxw:
                waits = list(si.on_wait)
                si.on_wait = waits[:maxw]
                extra = waits[maxw:]
                carriers = []
                for k in range(0, len(extra), maxw):
                    nop = mybir.InstNoOp(
                        name=f"{ins.name}-sw{k}",
                        engine=ins.engine,
                        bass_nofuse=True,
                        sync_info=mybir.SyncInfo(
                            on_wait=extra[k : k + maxw], on_update=[]
                        ),
                    )
                    carriers.append(nop)
                for j, nop in enumerate(carriers):
                    nc.register_instruction(nop, overwrite=True)
                    insts.insert(i + j, nop)
                i += len(carriers)
            i += 1


def _view(tile_ap, offset_ap, dims):
    """AP with explicit free dims; partition dim taken from tile_ap."""
    return bass.AP(offset_ap.tensor, offset_ap.offset, [tile_ap.ap[0]] + dims)


def build_graph(T, TD, has_bias, cfg=CFG):
    H, D, IN = cfg["H"], cfg["D"], cfg["IN"]
    HD = H * D
    DPC = cfg["N_DST"] // cfg["NCORES"]
    NW = (DPC + 127) // 128
    NSP = ((cfg["N_SRC"] + 127) // 128) * 128
    NSRC_T = NSP // 128
    inv_tem = 1.0 / cfg["TEM"]
    slope = cfg["SLOPE"]
    TT = T + TD

    nc = bass.Bass(num_swdge_queues=NQ, dynamic_dma_scratch_size=49152)
    f32, bf16, i16 = dt.float32, dt.bfloat16, dt.int16

    featT_src = nc.dram_tensor("featT_src", [IN, NSP], f32, kind="ExternalInput")
    featT_dst = nc.dram_tensor("featT_dst", [IN, NW * 128], f32, kind="ExternalInput")
    fc_w = nc.dram_tensor("fc_w", [IN, HD], f32, kind="ExternalInput")
    attn_l = nc.dram_tensor("attn_l", [H, D], f32, kind="ExternalInput")
    attn_r = nc.dram_tensor("attn_r", [H, D], f32, kind="ExternalInput")
    bias_in = nc.dram_tensor("bias", [1, HD], f32, kind="ExternalInput")
    prelu_in = nc.dram_tensor("prelu", [1, 1], f32, kind="ExternalInput")
    gm_idx = nc.dram_tensor("gm_idx", [128, NW * T * 8], i16, kind="ExternalInput")
    gd_idx = nc.dram_tensor("gd_idx", [128, NW * TD * 8], i16, kind="ExternalInput")
    mask_in = nc.dram_tensor("mask", [128, NW], f32, kind="ExternalInput")
    ident_in = nc.dram_tensor("ident_bf", [128, 128], bf16, kind="ExternalInput")
    oh_in = nc.dram_tensor("oh_all", [128, NW * TT * 128], bf16, kind="ExternalInput")
    ohT_in = nc.dram_tensor("ohT_all", [128, NW * TT * 128], bf16, kind="ExternalInput")

    hpos = nc.dram_tensor("hpos", [NW * 128, D], f32, kind="ExternalOutput")
    scal = nc.dram_tensor("scal", [1, 1], f32, kind="ExternalOutput")

    ztable = nc.dram_tensor("ztable", [NSP, ELEM], bf16)

    nc.gpsimd.load_library(library_config.mlp)

    with TileContext(nc) as tc:
        import contextlib

        ctx = contextlib.ExitStack()
        with ctx:
            cp = ctx.enter_context(tc.tile_pool(name="const", bufs=1))

            ident_bf = cp.tile([128, 128], bf16)
            nc.sync.dma_start(ident_bf[:], ident_in[:])
            gm_sb = cp.tile([128, NW * T * 8], i16)
            nc.sync.dma_start(gm_sb[:], gm_idx[:])
            gd_sb = cp.tile([128, NW * TD * 8], i16)
            nc.sync.dma_start(gd_sb[:], gd_idx[:])
            mask_sb = cp.tile([128, NW], f32)
            nc.sync.dma_start(mask_sb[:], mask_in[:])

            # fcwq = [fc_w | fc_w @ (WL|WR)] : one rhs for the fused phase-P matmul
            wlr = cp.tile([128, 2 * H], f32)
            nc.vector.memset(wlr[:], 0.0)
            for h in range(H):
                nc.sync.dma_start(
                    wlr[h * D : (h + 1) * D, h : h + 1],
                    bass.AP(attn_l, h * D, [[1, D], [1, 1]]),
                )
                nc.sync.dma_start(
                    wlr[h * D : (h + 1) * D, H + h : H + h + 1],
                    bass.AP(attn_r, h * D, [[1, D], [1, 1]]),
                )
            ones_row = cp.tile([1, 128], f32)
            nc.vector.memset(ones_row[:], 1.0)
            ones_col = cp.tile([128, 1], f32)
            nc.vector.memset(ones_col[:], 1.0)
            prelu_sb = cp.tile([1, 1], f32)
            nc.sync.dma_start(prelu_sb[:], prelu_in[:])

            fcwq = cp.tile([128, HD + 2 * H], f32)
            nc.sync.dma_start(fcwq[:, 0:HD], fc_w[:])
            bias_rep = cp.tile([128, HD], f32)
            prelu_bc = cp.tile([128, 1], f32)
            with tc.tile_pool(name="initps", bufs=1, space="PSUM") as ips:
                identf = cp.tile([128, 128], f32)
                nc.vector.tensor_copy(identf[:], ident_bf[:])
                fcwT_ps = ips.tile([128, HD], f32, tag="t")
                nc.tensor.transpose(fcwT_ps[:], fcwq[:, 0:HD], identf[:])
                fcwT = cp.tile([128, HD], f32)
                nc.vector.tensor_copy(fcwT[:], fcwT_ps[:])
                q_ps = ips.tile([128, 2 * H], f32, tag="t2")
                nc.tensor.matmul(q_ps[:], fcwT[:], wlr[:])
                nc.vector.tensor_copy(fcwq[:, HD : HD + 2 * H], q_ps[:])
                if has_bias:
                    bias_row = cp.tile([1, HD], f32)
                    nc.sync.dma_start(bias_row[:], bias_in[:])
                    br_ps = ips.tile([128, HD], f32, tag="t3")
                    nc.tensor.matmul(br_ps[:], ones_row[:], bias_row[:])
                    nc.vector.tensor_copy(bias_rep[:], br_ps[:])
                pr_ps = ips.tile([128, 1], f32, tag="t4")
                nc.tensor.matmul(pr_ps[:], ones_row[:], prelu_sb[:])
                nc.vector.tensor_copy(prelu_bc[:], pr_ps[:])

            zdst_pool = ctx.enter_context(tc.tile_pool(name="zdst", bufs=NW))
            elr_pool = ctx.enter_context(tc.tile_pool(name="elr", bufs=NW))
            erbf_pool = ctx.enter_context(tc.tile_pool(name="erbf", bufs=NW))
            zdst_t, elr_t, erbf_t = [], [], []

            # ---- Phase P ----
            PB = 3  # node tiles per psum batch (3*144 <= 512 f32 psum bank)
            LB = 12  # node tiles per input DMA
            HE = HD + 2 * H
            with (
                tc.tile_pool(name="pp", bufs=4) as pp,
                tc.tile_pool(name="pps", bufs=3, space="PSUM") as pps,
            ):
                def node_tile(src_ap):
                    ft = pp.tile([128, 128], f32, tag="ft")
                    nc.sync.dma_start(ft[:], src_ap)
                    ze_ps = pps.tile([128, HD + 2 * H], f32, tag="ze")
                    nc.tensor.matmul(ze_ps[:], ft[:], fcwq[:])
                    return ze_ps

                for l0 in range(0, NSRC_T, LB):
                    lb = min(LB, NSRC_T - l0)
                    ftb = pp.tile([128, LB * 128], f32, tag="ftb")
                    nc.sync.dma_start(
                        ftb[:, 0 : lb * 128],
                        featT_src[:, l0 * 128 : (l0 + lb) * 128],
                    )
                    for b0 in range(0, lb, PB):
                        pb = min(PB, lb - b0)
                        zeb = pps.tile([128, PB, HE], f32, tag="zeb")
                        for j in range(pb):
                            nc.tensor.matmul(
                                zeb[:, j, :],
                                ftb[:, (b0 + j) * 128 : (b0 + j + 1) * 128],
                                fcwq[:],
                            )
                        zt = pp.tile([128, PB, HD + H], bf16, tag="zt")
                        ze_v = _view(
                            zeb[:], zeb[:, 0, 0 : HD + H], [[HE, pb], [1, HD + H]]
                        )
                        nc.vector.tensor_copy(zt[:, 0:pb, :], ze_v)
                        dst = bass.AP(
                            ztable, (l0 + b0) * 128 * ELEM,
                            [[ELEM, 128], [128 * ELEM, pb], [1, HD + H]],
                        )
                        nc.scalar.dma_start(dst, zt[:, 0:pb, :])

                for w in range(NW):
                    ze_ps = node_tile(featT_dst[:, w * 128 : (w + 1) * 128])
                    zd = zdst_pool.tile([128, HD], f32)
                    nc.vector.tensor_copy(zd[:], ze_ps[:, 0:HD])
                    el = elr_pool.tile([128, 2 * H], f32)
                    nc.vector.tensor_copy(el[:], ze_ps[:, HD : HD + 2 * H])
                    eb = erbf_pool.tile([128, H], bf16)
                    nc.vector.tensor_copy(eb[:], ze_ps[:, HD + H : HD + 2 * H])
                    zdst_t.append(zd)
                    elr_t.append(el)
                    erbf_t.append(eb)

            # ---- Phase E ----
            ep = ctx.enter_context(tc.tile_pool(name="ep", bufs=3))
            gpool = ctx.enter_context(tc.tile_pool(name="gp", bufs=6))
            finp = ctx.enter_context(tc.tile_pool(name="finp", bufs=3))
            ps_er = ctx.enter_context(tc.tile_pool(name="psE", bufs=2, space="PSUM"))
            ps_sm = ctx.enter_context(tc.tile_pool(name="psM", bufs=2, space="PSUM"))
            ps_sd = ctx.enter_context(tc.tile_pool(name="psD", bufs=2, space="PSUM"))

            nm_buf = cp.tile([128, NW], f32)
            na_buf = cp.tile([128, NW], f32)
            nb_buf = cp.tile([128, NW], f32)

            st = {}

            def stage_gather(w):
                gm = gpool.tile([128, T, ELEM], bf16, tag="gm")
                nc.gpsimd.dma_gather(
                    gm[:], ztable[:], gm_sb[:, w * T * 8 : (w + 1) * T * 8],
                    T * 128, T * 128, ELEM, single_packet=False,
                    queue_num=w % NQ,
                )
                gd = gpool.tile([128, TD, ELEM], bf16, tag="gd")
                nc.gpsimd.dma_gather(
                    gd[:], ztable[:], gd_sb[:, w * TD * 8 : (w + 1) * TD * 8],
                    TD * 128, TD * 128, ELEM, single_packet=False,
                    queue_num=(w + 2) % NQ,
                )
                oh = gpool.tile([128, TT, 128], bf16, tag="oh")
                nc.sync.dma_start(
                    oh[:], oh_in[:, w * TT * 128 : (w + 1) * TT * 128]
                )
                ohT = gpool.tile([128, TT * 128], bf16, tag="ohTs")
                nc.scalar.dma_start(
                    ohT[:], ohT_in[:, w * TT * 128 : (w + 1) * TT * 128]
                )
                st[w] = dict(gm=gm, gd=gd, oh=oh, ohT=ohT)

            def stage_compute(w):
                s = st[w]
                gm, gd, oh, ohT = s["gm"], s["gd"], s["oh"], s["ohT"]
                er_ps = ps_er.tile([128, TT * H], f32, tag="er")
                for t in range(TT):
                    nc.tensor.matmul(
                        er_ps[:, t * H : (t + 1) * H],
                        ohT[:, t * 128 : (t + 1) * 128],
                        erbf_t[w][:],
                    )
                ew = ep.tile([128, TT * H], f32, tag="ew")
                el_m = _view(gm[:], gm[:, 0, HD : HD + H], [[ELEM, T], [1, H]])
                nc.vector.scalar_tensor_tensor(
                    ew[:, 0 : T * H], er_ps[:, 0 : T * H], 1.0, el_m, AT.mult, AT.add
                )
                el_d = _view(gd[:], gd[:, 0, HD : HD + H], [[ELEM, TD], [1, H]])
                nc.vector.scalar_tensor_tensor(
                    ew[:, T * H : TT * H], er_ps[:, T * H : TT * H], 1.0, el_d,
                    AT.mult, AT.add,
                )
                lk = ep.tile([128, TT * H], f32, tag="lk")
                nc.vector.scalar_tensor_tensor(
                    lk[:], ew[:], slope, ew[:], AT.mult, AT.max
                )
                ms = ep.tile([128, TT, HD + H], bf16, tag="ms")
                al_out = _view(ms[:], ms[:, 0, HD : HD + H], [[HD + H, TT], [1, H]])
                nc.scalar.activation(al_out, lk[:], AF.Exp)
                al_exp = ep.tile([128, TT, HD], bf16, tag="alx")
                lk_v = _view(lk[:], lk[:], [[1, TT * H], [0, D]])
                nc.scalar.activation(al_exp[:], lk_v, AF.Exp)
                for t in range(T):
                    nc.vector.tensor_tensor(
                        ms[:, t, 0:HD], gm[:, t, 0:HD], al_exp[:, t, :], AT.mult
                    )
                for t in range(TD):
                    nc.vector.tensor_tensor(
                        ms[:, T + t, 0:HD], gd[:, t, 0:HD], al_exp[:, T + t, :],
                        AT.mult,
                    )
                seg_m = ps_sm.tile([128, HD + H], f32, tag="sm")
                seg_d = ps_sd.tile([128, HD + H], f32, tag="sd")
                for t in range(TT):
                    tgt = seg_m if t < T else seg_d
                    nc.tensor.matmul(
                        tgt[:], oh[:, t, :], ms[:, t, :],
                        start=(t == 0 or t == T), stop=(t == T - 1 or t == TT - 1),
                    )
                s["seg_m"] = seg_m
                s["seg_d"] = seg_d

            def stage_final(w):
                s = st.pop(w)
                seg_m, seg_d = s["seg_m"], s["seg_d"]
                es = finp.tile([128, H], f32, tag="es")
                nc.vector.tensor_tensor(
                    es[:], elr_t[w][:, 0:H], elr_t[w][:, H : 2 * H], AT.add
                )
                nc.vector.scalar_tensor_tensor(es[:], es[:], slope, es[:], AT.mult, AT.max)
                ws_ = finp.tile([128, H], f32, tag="ws")
                nc.scalar.activation(ws_[:], es[:], AF.Exp)

                dp = finp.tile([128, H], f32, tag="dp")
                nc.vector.tensor_tensor(dp[:], seg_m[:, HD : HD + H], ws_[:], AT.add)
                dn = finp.tile([128, H], f32, tag="dn")
                nc.vector.tensor_tensor(dn[:], dp[:], seg_d[:, HD : HD + H], AT.subtract)
                nc.vector.tensor_scalar(dp[:], dp[:], float(H), None, AT.mult)
                nc.vector.tensor_scalar(dn[:], dn[:], float(H), None, AT.mult)
                rp = finp.tile([128, H], f32, tag="rp")
                nc.vector.reciprocal(rp[:], dp[:])
                rn = finp.tile([128, H], f32, tag="rn")
                nc.vector.reciprocal(rn[:], dn[:])

                ws_e = finp.tile([128, HD], f32, tag="wse")
                ws_v = _view(ws_[:], ws_[:, 0:H], [[1, H], [0, D]])
                nc.scalar.copy(ws_e[:], ws_v)
                rp_e = finp.tile([128, HD], f32, tag="rpe")
                rp_v = _view(rp[:], rp[:, 0:H], [[1, H], [0, D]])
                nc.scalar.copy(rp_e[:], rp_v)
                rn_e = finp.tile([128, HD], f32, tag="rne")
                rn_v = _view(rn[:], rn[:, 0:H], [[1, H], [0, D]])
                nc.scalar.copy(rn_e[:], rn_v)
                sm_ = finp.tile([128, HD], f32, tag="smv")
                nc.vector.tensor_tensor(sm_[:], zdst_t[w][:], ws_e[:], AT.mult)
                rstp_r = finp.tile([128, HD], f32, tag="rpr")
                nc.vector.tensor_tensor(rstp_r[:], seg_m[:, 0:HD], sm_[:], AT.add)
                rstn_r = finp.tile([128, HD], f32, tag="rnr")
                nc.vector.tensor_tensor(rstn_r[:], rstp_r[:], seg_d[:, 0:HD], AT.subtract)

                rstp = finp.tile([128, HD], f32, tag="rstp")
                nc.vector.tensor_tensor(rstp[:], rstp_r[:], rp_e[:], AT.mult)
                rstn = finp.tile([128, HD], f32, tag="rstn")
                nc.vector.tensor_tensor(rstn[:], rstn_r[:], rn_e[:], AT.mult)

                if has_bias:
                    nc.vector.tensor_tensor(rstp[:], rstp[:], bias_rep[:], AT.add)
                    nc.vector.tensor_tensor(rstn[:], rstn[:], bias_rep[:], AT.add)
                nc.vector.scalar_tensor_tensor(
                    rstp[:], rstp[:], prelu_bc[:, 0:1], rstp[:], AT.mult, AT.max
                )
                nc.vector.scalar_tensor_tensor(
                    rstn[:], rstn[:], prelu_bc[:, 0:1], rstn[:], AT.mult, AT.max
                )

                h8p = finp.tile([128, D], f32, tag="h8p")
                nc.vector.tensor_reduce(
                    h8p[:], _view(rstp[:], rstp[:, 0:HD], [[1, D], [D, H]]),
                    AX.X, AT.add,
                )
                h8n = finp.tile([128, D], f32, tag="h8n")
                nc.vector.tensor_reduce(
                    h8n[:], _view(rstn[:], rstn[:, 0:HD], [[1, D], [D, H]]),
                    AX.X, AT.add,
                )
                nc.scalar.dma_start(hpos[w * 128 : (w + 1) * 128, :], h8p[:])

                scr = finp.tile([128, D], f32, tag="scr")
                nc.vector.tensor_tensor(scr[:], h8p[:], h8n[:], AT.mult)
                nc.vector.tensor_reduce(nm_buf[:, w : w + 1], scr[:], AX.X, AT.add)
                nc.vector.tensor_tensor(scr[:], h8p[:], h8p[:], AT.mult)
                nc.vector.tensor_reduce(na_buf[:, w : w + 1], scr[:], AX.X, AT.add)
                nc.vector.tensor_tensor(scr[:], h8n[:], h8n[:], AT.mult)
                nc.vector.tensor_reduce(nb_buf[:, w : w + 1], scr[:], AX.X, AT.add)

            # skewed software pipeline: gather(w) || compute(w-1) || final(w-2)
            for w in range(NW + 2):
                if w < NW:
                    stage_gather(w)
                if 1 <= w < NW + 1:
                    stage_compute(w - 1)
                if w >= 2:
                    stage_final(w - 2)

            # ---- batched cosine / LSE tail ----
            pr2 = cp.tile([128, NW], f32)
            nc.vector.tensor_tensor(pr2[:], na_buf[:], nb_buf[:], AT.mult)
            nc.vector.tensor_scalar(pr2[:], pr2[:], 1e-30, None, AT.max)
            sq = cp.tile([128, NW], f32)
            nc.scalar.activation(sq[:], pr2[:], AF.Sqrt)
            rr = cp.tile([128, NW], f32)
            nc.vector.reciprocal(rr[:], sq[:])
            cosv = cp.tile([128, NW], f32)
            nc.vector.tensor_tensor(cosv[:], nm_buf[:], rr[:], AT.mult)
            ctr = cp.tile([128, NW], f32)
            nc.scalar.activation(ctr[:], cosv[:], AF.Exp, scale=inv_tem)
            nc.vector.tensor_tensor(ctr[:], ctr[:], mask_sb[:], AT.mult)
            acc = cp.tile([128, 1], f32)
            nc.vector.tensor_reduce(acc[:], ctr[:], AX.X, AT.add)
            with tc.tile_pool(name="fps", bufs=1, space="PSUM") as fps:
                tot_ps = fps.tile([1, 1], f32)
                nc.tensor.matmul(tot_ps[:], acc[:], ones_col[:])
                tot = cp.tile([1, 1], f32)
                nc.vector.tensor_copy(tot[:], tot_ps[:])
                nc.sync.dma_start(scal[:], tot[:])

    library_overlay.lower_extended_insts(nc)
    _split_sync_waits(nc)
    return nc


# ---------------------------------------------------------------------------
# host side
# ---------------------------------------------------------------------------

_GRAPH_CACHE = {}


def prep_inputs(feat, fc_w, attn_l, attn_r, bias, prelu_a, edge_src, edge_dst,
                neg_idx, cfg=CFG):
    NS, ND, E = cfg["N_SRC"], cfg["N_DST"], cfg["E"]
    NC_, H, D, IN = cfg["NCORES"], cfg["H"], cfg["D"], cfg["IN"]
    DPC = ND // NC_
    NW = (DPC + 127) // 128
    NSP = ((NS + 127) // 128) * 128

    feat = np.asarray(feat, np.float32)
    edge_src = np.asarray(edge_src, np.int64)
    edge_dst = np.asarray(edge_dst, np.int64)
    neg_idx = np.asarray(neg_idx, np.int64)

    keep = np.zeros(E, bool)
    keep[neg_idx] = True

    order = np.argsort(edge_dst, kind="stable")
    src_s = edge_src[order]
    dst_s = edge_dst[order]
    drop_s = ~keep[order]

    ld = dst_s - NS
    core = ld // DPC
    lw = (ld % DPC) // 128
    rel = (ld % DPC) % 128

    key = core * NW + lw
    main_counts = np.bincount(key, minlength=NC_ * NW)
    drop_counts = np.bincount(key[drop_s], minlength=NC_ * NW)
    T = max(1, int(-(-main_counts.max() // 128)))
    TD = max(1, int(-(-drop_counts.max() // 128)))

    def build_imgs(sel_src, sel_key, sel_rel, TL):
        o2 = np.argsort(sel_key, kind="stable")
        ss, kk, rr = sel_src[o2], sel_key[o2], sel_rel[o2]
        starts = np.searchsorted(kk, np.arange(NC_ * NW))
        ends = np.searchsorted(kk, np.arange(NC_ * NW) + 1)
        idx_imgs = np.zeros((NC_, 128, NW * TL * 8), np.int16)
        rel_imgs = np.full((NC_, 128, NW * TL), -1.0, np.float32)
        npad = TL * 128
        for c in range(NC_):
            for w in range(NW):
                k = c * NW + w
                s, e = starts[k], ends[k]
                n = e - s
                idx = np.zeros(npad, np.int16)
                idx[:n] = ss[s:e].astype(np.int16)
                rl = np.full(npad, -1.0, np.float32)
                rl[:n] = rr[s:e].astype(np.float32)
                img16 = idx.reshape(TL * 8, 16).T
                idx_imgs[c][:, w * TL * 8 : (w + 1) * TL * 8] = np.tile(img16, (8, 1))
                rel_imgs[c][:, w * TL : (w + 1) * TL] = rl.reshape(TL, 128).T
        return idx_imgs, rel_imgs

    gm_imgs, relm_imgs = build_imgs(src_s, key, rel, T)
    gd_imgs, reld_imgs = build_imgs(src_s[drop_s], key[drop_s], rel[drop_s], TD)
    TT = T + TD

    featT_src = np.zeros((IN, NSP), np.float32)
    featT_src[:, :NS] = feat[:NS].T

    p = np.arange(128)[:, None]
    k = np.arange(NW)[None, :]
    mask = ((k * 128 + p) < DPC).astype(np.float32)

    try:
        import ml_dtypes
        bfdt = ml_dtypes.bfloat16
    except ImportError:
        import jax.numpy as jnp
        bfdt = jnp.bfloat16
    ident_bf = np.eye(128, dtype=np.float32).astype(bfdt)
    ar128 = np.arange(128, dtype=np.float32)

    def build_onehots(relm_c, reld_c):
        # rel images: [128(e), NW*TL]; returns oh/ohT [128, NW*TT*128] bf16
        rm = relm_c.reshape(128, NW, T)
        rd = reld_c.reshape(128, NW, TD)
        rel_all = np.concatenate([rm, rd], axis=2)  # [128, NW, TT]
        oh = (rel_all[:, :, :, None] == ar128[None, None, None, :])
        ohb = oh.astype(bfdt).reshape(128, NW * TT * 128)
        # ohT[d, w, t, e] = (rel_all[e, w, t] == d)
        ohT = (rel_all.transpose(1, 2, 0)[None, :, :, :] ==
               ar128[:, None, None, None])
        ohTb = ohT.astype(bfdt).reshape(128, NW * TT * 128)
        return ohb, ohTb

    has_bias = bool(np.any(np.asarray(bias)))

    in_maps = []
    for c in range(NC_):
        oh_c, ohT_c = build_onehots(relm_imgs[c], reld_imgs[c])
        fdT = np.zeros((IN, NW * 128), np.float32)
        fdT[:, :DPC] = feat[NS + c * DPC : NS + (c + 1) * DPC].T
        in_maps.append(
            dict(
                featT_src=featT_src,
                featT_dst=fdT,
                fc_w=np.asarray(fc_w, np.float32),
                attn_l=np.asarray(attn_l, np.float32),
                attn_r=np.asarray(attn_r, np.float32),
                bias=np.asarray(bias, np.float32).reshape(1, -1),
                prelu=np.asarray(prelu_a, np.float32).reshape(1, 1),
                gm_idx=gm_imgs[c],
                gd_idx=gd_imgs[c],
                mask=mask,
                ident_bf=ident_bf,
                oh_all=oh_c,
                ohT_all=ohT_c,
            )
        )
    return in_maps, T, TD, has_bias


def run(inputs, trace=False, cfg=CFG):
    in_maps, T, TD, has_bias = prep_inputs(**inputs, cfg=cfg)
    ck = (T, TD, has_bias, id(cfg) if cfg is not CFG else 0)
    if ck not in _GRAPH_CACHE:
        _GRAPH_CACHE[ck] = build_graph(T, TD, has_bias, cfg)
    nc = _GRAPH_CACHE[ck]
    if trace:
        import antenv.axon_hooks as ah
        ah.register_default()
    res = run_bass_kernel_spmd(
        nc, in_maps, core_ids=list(range(cfg["NCORES"])), trace=trace
    )
    NC_, ND = cfg["NCORES"], cfg["N_DST"]
    DPC = ND // NC_
    hp = np.concatenate([res.results[c]["hpos"][:DPC] for c in range(NC_)], axis=0)
    s = np.sum([np.float64(res.results[c]["scal"][0, 0]) for c in range(NC_)])
    loss = np.float32(np.log(s))
    return (np.asarray(loss, np.float32), hp), res


def kernel(**inputs):
    out, _res = run(inputs, trace=False)
    return out


# revision 20
# speedup vs baseline: 1.3395x; 1.1388x over previous
"""BiGraphContrastLayer (GAT + drop-edge contrast) on 8 TRN2 NeuronCores.

Strategy: dst-node partitioning (2500 dst nodes per core, no collectives).
 - Phase P (per core, replicated src work): z = feat @ fc_w, el/er attention
   logits; src rows packed into a bf16 DRAM gather table [z(128) | el(8)].
   feat arrives host-pretransposed so no on-device transposes are needed;
   one fused matmul per 128-node tile computes z and el|er together.
 - Phase E: per 128-dst window, dma_gather z rows by edge_src (4 SWDGE
   queues round-robin: descriptor processing is the gather bottleneck and
   parallelizes across queues), batched one-hot build, per-tile PE
   transposes for the er-expansion matmuls, segment-softmax (shift m=0 is
   exact: logits are O(1)) and weighted segment-sums via TensorE matmuls
   accumulating in PSUM. The negative graph (1% of edges dropped) is
   aggregated as pos - dropped, so gathers are shared between graphs.
 - Finalize: self-loop term, normalize (x8 folded in for the head-mean),
   PReLU, head-mean, then a batched cosine/LSE tail. Host takes log of the
   summed per-core partials and concatenates h_pos shards.
"""

import sys
import numpy as np

sys.path.insert(0, "/opt/trn_rl_repo")

import antenv  # noqa: E402

if "/opt/trn_rl_repo/antenv" not in antenv.__path__:
    antenv.__path__.append("/opt/trn_rl_repo/antenv")

import concourse.bass as bass  # noqa: E402
import concourse.mybir as mybir  # noqa: E402
from concourse import library_config  # noqa: E402
from concourse import library_overlay  # noqa: E402
from concourse.tile import TileContext  # noqa: E402
from concourse.bass_utils import run_bass_kernel_spmd  # noqa: E402

dt = mybir.dt
AT = mybir.AluOpType
AF = mybir.ActivationFunctionType
AX = mybir.AxisListType

CFG = dict(
    N_SRC=20000,
    N_DST=20000,
    E=320000,
    H=8,
    D=16,
    IN=128,
    NCORES=8,
    TEM=0.7,
    SLOPE=0.2,
)
ELEM = 256  # bf16 elems per table row (512B): z[0:128], el[128:136], pad
NQ = 4  # SWDGE queues for gathers

MAX_SYNC_WAITS = 1


def _split_sync_waits(nc, maxw=MAX_SYNC_WAITS):
    """walrus here rejects >~2 sync waits per instruction; split extras onto
    InstNoOp carriers inserted before, same engine (stream order is kept)."""
    for _name, handle in nc.bb_map.items():
        bb = handle.bb
        insts = bb.instructions
        i = 0
        while i < len(insts):
            ins = insts[i]
            si = ins.sync_info
            if si is not None and si.on_wait and len(si.on_wait) > maxw:
                waits = list(si.on_wait)
                si.on_wait = waits[:maxw]
                extra = waits[maxw:]
                carriers = []
                for k in range(0, len(extra), maxw):
                    nop = mybir.InstNoOp(
                        name=f"{ins.name}-sw{k}",
                        engine=ins.engine,
                        bass_nofuse=True,
                        sync_info=mybir.SyncInfo(
                            on_wait=extra[k : k + maxw], on_update=[]
                        ),
                    )
                    carriers.append(nop)
                for j, nop in enumerate(carriers):
                    nc.register_instruction(nop, overwrite=True)
                    insts.insert(i + j, nop)
                i += len(carriers)
            i += 1


def _view(tile_ap, offset_ap, dims):
    """AP with explicit free dims; partition dim taken from tile_ap."""
    return bass.AP(offset_ap.tensor, offset_ap.offset, [tile_ap.ap[0]] + dims)


def build_graph(T, TD, has_bias, cfg=CFG):
    H, D, IN = cfg["H"], cfg["D"], cfg["IN"]
    HD = H * D
    DPC = cfg["N_DST"] // cfg["NCORES"]
    NW = (DPC + 127) // 128
    NSP = ((cfg["N_SRC"] + 127) // 128) * 128
    NSRC_T = NSP // 128
    inv_tem = 1.0 / cfg["TEM"]
    slope = cfg["SLOPE"]
    TT = T + TD

    nc = bass.Bass(num_swdge_queues=NQ, dynamic_dma_scratch_size=49152)
    f32, bf16, i16 = dt.float32, dt.bfloat16, dt.int16

    featT_src = nc.dram_tensor("featT_src", [IN, NSP], f32, kind="ExternalInput")
    featT_dst = nc.dram_tensor("featT_dst", [IN, NW * 128], f32, kind="ExternalInput")
    fc_w = nc.dram_tensor("fc_w", [IN, HD], f32, kind="ExternalInput")
    attn_l = nc.dram_tensor("attn_l", [H, D], f32, kind="ExternalInput")
    attn_r = nc.dram_tensor("attn_r", [H, D], f32, kind="ExternalInput")
    bias_in = nc.dram_tensor("bias", [1, HD], f32, kind="ExternalInput")
    prelu_in = nc.dram_tensor("prelu", [1, 1], f32, kind="ExternalInput")
    gm_idx = nc.dram_tensor("gm_idx", [128, NW * T * 8], i16, kind="ExternalInput")
    gd_idx = nc.dram_tensor("gd_idx", [128, NW * TD * 8], i16, kind="ExternalInput")
    mask_in = nc.dram_tensor("mask", [128, NW], f32, kind="ExternalInput")
    ident_in = nc.dram_tensor("ident_bf", [128, 128], bf16, kind="ExternalInput")
    oh_in = nc.dram_tensor("oh_all", [128, NW * TT * 128], dt.float8e4, kind="ExternalInput")
    ohT_in = nc.dram_tensor("ohT_all", [128, NW * TT * 128], dt.float8e4, kind="ExternalInput")

    hpos = nc.dram_tensor("hpos", [NW * 128, D], f32, kind="ExternalOutput")
    scal = nc.dram_tensor("scal", [1, 1], f32, kind="ExternalOutput")

    ztable = nc.dram_tensor("ztable", [NSP, ELEM], bf16)

    nc.gpsimd.load_library(library_config.mlp)

    with TileContext(nc) as tc:
        import contextlib

        ctx = contextlib.ExitStack()
        with ctx:
            cp = ctx.enter_context(tc.tile_pool(name="const", bufs=1))

            ident_bf = cp.tile([128, 128], bf16)
            nc.sync.dma_start(ident_bf[:], ident_in[:])
            gm_sb = cp.tile([128, NW * T * 8], i16)
            nc.sync.dma_start(gm_sb[:], gm_idx[:])
            gd_sb = cp.tile([128, NW * TD * 8], i16)
            nc.sync.dma_start(gd_sb[:], gd_idx[:])
            mask_sb = cp.tile([128, NW], f32)
            nc.sync.dma_start(mask_sb[:], mask_in[:])

            # fcwq = [fc_w | fc_w @ (WL|WR)] : one rhs for the fused phase-P matmul
            wlr = cp.tile([128, 2 * H], f32)
            nc.vector.memset(wlr[:], 0.0)
            for h in range(H):
                nc.sync.dma_start(
                    wlr[h * D : (h + 1) * D, h : h + 1],
                    bass.AP(attn_l, h * D, [[1, D], [1, 1]]),
                )
                nc.sync.dma_start(
                    wlr[h * D : (h + 1) * D, H + h : H + h + 1],
                    bass.AP(attn_r, h * D, [[1, D], [1, 1]]),
                )
            ones_row = cp.tile([1, 128], f32)
            nc.vector.memset(ones_row[:], 1.0)
            ones_col = cp.tile([128, 1], f32)
            nc.vector.memset(ones_col[:], 1.0)
            prelu_sb = cp.tile([1, 1], f32)
            nc.sync.dma_start(prelu_sb[:], prelu_in[:])

            fcwq = cp.tile([128, HD + 2 * H], f32)
            nc.sync.dma_start(fcwq[:, 0:HD], fc_w[:])
            bias_rep = cp.tile([128, HD], f32)
            prelu_bc = cp.tile([128, 1], f32)
            with tc.tile_pool(name="initps", bufs=1, space="PSUM") as ips:
                identf = cp.tile([128, 128], f32)
                nc.vector.tensor_copy(identf[:], ident_bf[:])
                fcwT_ps = ips.tile([128, HD], f32, tag="t")
                nc.tensor.transpose(fcwT_ps[:], fcwq[:, 0:HD], identf[:])
                fcwT = cp.tile([128, HD], f32)
                nc.vector.tensor_copy(fcwT[:], fcwT_ps[:])
                q_ps = ips.tile([128, 2 * H], f32, tag="t2")
                nc.tensor.matmul(q_ps[:], fcwT[:], wlr[:])
                nc.vector.tensor_copy(fcwq[:, HD : HD + 2 * H], q_ps[:])
                if has_bias:
                    bias_row = cp.tile([1, HD], f32)
                    nc.sync.dma_start(bias_row[:], bias_in[:])
                    br_ps = ips.tile([128, HD], f32, tag="t3")
                    nc.tensor.matmul(br_ps[:], ones_row[:], bias_row[:])
                    nc.vector.tensor_copy(bias_rep[:], br_ps[:])
                pr_ps = ips.tile([128, 1], f32, tag="t4")
                nc.tensor.matmul(pr_ps[:], ones_row[:], prelu_sb[:])
                nc.vector.tensor_copy(prelu_bc[:], pr_ps[:])

            zdst_pool = ctx.enter_context(tc.tile_pool(name="zdst", bufs=NW))
            elr_pool = ctx.enter_context(tc.tile_pool(name="elr", bufs=NW))
            erbf_pool = ctx.enter_context(tc.tile_pool(name="erbf", bufs=NW))
            zdst_t, elr_t, erbf_t = [], [], []

            # ---- Phase P ----
            PB = 3  # node tiles per psum batch (3*144 <= 512 f32 psum bank)
            LB = 12  # node tiles per input DMA
            HE = HD + 2 * H
            with (
                tc.tile_pool(name="pp", bufs=4) as pp,
                tc.tile_pool(name="pps", bufs=3, space="PSUM") as pps,
            ):
                def node_tile(src_ap):
                    ft = pp.tile([128, 128], f32, tag="ft")
                    nc.sync.dma_start(ft[:], src_ap)
                    ze_ps = pps.tile([128, HD + 2 * H], f32, tag="ze")
                    nc.tensor.matmul(ze_ps[:], ft[:], fcwq[:])
                    return ze_ps

                for l0 in range(0, NSRC_T, LB):
                    lb = min(LB, NSRC_T - l0)
                    ftb = pp.tile([128, LB * 128], f32, tag="ftb")
                    nc.sync.dma_start(
                        ftb[:, 0 : lb * 128],
                        featT_src[:, l0 * 128 : (l0 + lb) * 128],
                    )
                    for b0 in range(0, lb, PB):
                        pb = min(PB, lb - b0)
                        zeb = pps.tile([128, PB, HE], f32, tag="zeb")
                        for j in range(pb):
                            nc.tensor.matmul(
                                zeb[:, j, :],
                                ftb[:, (b0 + j) * 128 : (b0 + j + 1) * 128],
                                fcwq[:],
                            )
                        zt = pp.tile([128, PB, HD + H], bf16, tag="zt")
                        ze_v = _view(
                            zeb[:], zeb[:, 0, 0 : HD + H], [[HE, pb], [1, HD + H]]
                        )
                        nc.vector.tensor_copy(zt[:, 0:pb, :], ze_v)
                        dst = bass.AP(
                            ztable, (l0 + b0) * 128 * ELEM,
                            [[ELEM, 128], [128 * ELEM, pb], [1, HD + H]],
                        )
                        nc.scalar.dma_start(dst, zt[:, 0:pb, :])

                for w in range(NW):
                    ze_ps = node_tile(featT_dst[:, w * 128 : (w + 1) * 128])
                    zd = zdst_pool.tile([128, HD], f32)
                    nc.vector.tensor_copy(zd[:], ze_ps[:, 0:HD])
                    el = elr_pool.tile([128, 2 * H], f32)
                    nc.vector.tensor_copy(el[:], ze_ps[:, HD : HD + 2 * H])
                    eb = erbf_pool.tile([128, H], bf16)
                    nc.vector.tensor_copy(eb[:], ze_ps[:, HD + H : HD + 2 * H])
                    zdst_t.append(zd)
                    elr_t.append(el)
                    erbf_t.append(eb)

            # ---- Phase E ----
            ep = ctx.enter_context(tc.tile_pool(name="ep", bufs=3))
            gpool = ctx.enter_context(tc.tile_pool(name="gp", bufs=6))
            finp = ctx.enter_context(tc.tile_pool(name="finp", bufs=3))
            ps_er = ctx.enter_context(tc.tile_pool(name="psE", bufs=2, space="PSUM"))
            ps_sm = ctx.enter_context(tc.tile_pool(name="psM", bufs=2, space="PSUM"))
            ps_sd = ctx.enter_context(tc.tile_pool(name="psD", bufs=2, space="PSUM"))

            nm_buf = cp.tile([128, NW], f32)
            na_buf = cp.tile([128, NW], f32)
            nb_buf = cp.tile([128, NW], f32)

            st = {}
            r_t1 = nc.gpsimd.to_reg(((T + 1) // 2) * 128)
            r_t2 = nc.gpsimd.to_reg((T - (T + 1) // 2) * 128)
            r_td = nc.gpsimd.to_reg(TD * 128)

            def stage_gather(w):
                gm = gpool.tile([128, T, ELEM], bf16, tag="gm")
                T1 = (T + 1) // 2
                T2 = T - T1
                nc.gpsimd.dma_gather(
                    gm[:, 0:T1, :], ztable[:],
                    gm_sb[:, w * T * 8 : w * T * 8 + T1 * 8],
                    T1 * 128, r_t1, ELEM, single_packet=False,
                    queue_num=(3 * w) % NQ,
                )
                nc.gpsimd.dma_gather(
                    gm[:, T1:T, :], ztable[:],
                    gm_sb[:, w * T * 8 + T1 * 8 : (w + 1) * T * 8],
                    T2 * 128, r_t2, ELEM, single_packet=False,
                    queue_num=(3 * w + 1) % NQ,
                )
                gd = gpool.tile([128, TD, ELEM], bf16, tag="gd")
                nc.gpsimd.dma_gather(
                    gd[:], ztable[:], gd_sb[:, w * TD * 8 : (w + 1) * TD * 8],
                    TD * 128, r_td, ELEM, single_packet=False,
                    queue_num=(3 * w + 2) % NQ,
                )
                oh = gpool.tile([128, TT, 128], dt.float8e4, tag="oh")
                nc.sync.dma_start(
                    oh[:], oh_in[:, w * TT * 128 : (w + 1) * TT * 128]
                )
                ohT = gpool.tile([128, TT * 128], dt.float8e4, tag="ohTs")
                nc.scalar.dma_start(
                    ohT[:], ohT_in[:, w * TT * 128 : (w + 1) * TT * 128]
                )
                st[w] = dict(gm=gm, gd=gd, oh=oh, ohT=ohT)

            def stage_compute(w):
                s = st[w]
                gm, gd, oh, ohT = s["gm"], s["gd"], s["oh"], s["ohT"]
                er_ps = ps_er.tile([128, TT * H], f32, tag="er")
                for t in range(TT):
                    nc.tensor.matmul(
                        er_ps[:, t * H : (t + 1) * H],
                        ohT[:, t * 128 : (t + 1) * 128],
                        erbf_t[w][:],
                    )
                ew = ep.tile([128, TT * H], f32, tag="ew")
                el_m = _view(gm[:], gm[:, 0, HD : HD + H], [[ELEM, T], [1, H]])
                nc.vector.scalar_tensor_tensor(
                    ew[:, 0 : T * H], er_ps[:, 0 : T * H], 1.0, el_m, AT.mult, AT.add
                )
                el_d = _view(gd[:], gd[:, 0, HD : HD + H], [[ELEM, TD], [1, H]])
                nc.vector.scalar_tensor_tensor(
                    ew[:, T * H : TT * H], er_ps[:, T * H : TT * H], 1.0, el_d,
                    AT.mult, AT.add,
                )
                lk = ep.tile([128, TT * H], f32, tag="lk")
                nc.vector.scalar_tensor_tensor(
                    lk[:], ew[:], slope, ew[:], AT.mult, AT.max
                )
                ms = ep.tile([128, TT, HD + H], bf16, tag="ms")
                al_out = _view(ms[:], ms[:, 0, HD : HD + H], [[HD + H, TT], [1, H]])
                nc.scalar.activation(al_out, lk[:], AF.Exp)
                al_exp = ep.tile([128, TT, HD], bf16, tag="alx")
                lk_v = _view(lk[:], lk[:], [[1, TT * H], [0, D]])
                nc.scalar.activation(al_exp[:], lk_v, AF.Exp)
                for t in range(T):
                    nc.vector.tensor_tensor(
                        ms[:, t, 0:HD], gm[:, t, 0:HD], al_exp[:, t, :], AT.mult
                    )
                for t in range(TD):
                    nc.vector.tensor_tensor(
                        ms[:, T + t, 0:HD], gd[:, t, 0:HD], al_exp[:, T + t, :],
                        AT.mult,
                    )
                seg_m = ps_sm.tile([128, HD + H], f32, tag="sm")
                seg_d = ps_sd.tile([128, HD + H], f32, tag="sd")
                for t in range(TT):
                    tgt = seg_m if t < T else seg_d
                    nc.tensor.matmul(
                        tgt[:], oh[:, t, :], ms[:, t, :],
                        start=(t == 0 or t == T), stop=(t == T - 1 or t == TT - 1),
                    )
                s["seg_m"] = seg_m
                s["seg_d"] = seg_d

            def stage_final(w):
                s = st.pop(w)
                seg_m, seg_d = s["seg_m"], s["seg_d"]
                es = finp.tile([128, H], f32, tag="es")
                nc.vector.tensor_tensor(
                    es[:], elr_t[w][:, 0:H], elr_t[w][:, H : 2 * H], AT.add
                )
                nc.vector.scalar_tensor_tensor(es[:], es[:], slope, es[:], AT.mult, AT.max)
                ws_ = finp.tile([128, H], f32, tag="ws")
                nc.scalar.activation(ws_[:], es[:], AF.Exp)

                dp = finp.tile([128, H], f32, tag="dp")
                nc.vector.tensor_tensor(dp[:], seg_m[:, HD : HD + H], ws_[:], AT.add)
                dn = finp.tile([128, H], f32, tag="dn")
                nc.vector.tensor_tensor(dn[:], dp[:], seg_d[:, HD : HD + H], AT.subtract)
                nc.vector.tensor_scalar(dp[:], dp[:], float(H), None, AT.mult)
                nc.vector.tensor_scalar(dn[:], dn[:], float(H), None, AT.mult)
                rp = finp.tile([128, H], f32, tag="rp")
                nc.vector.reciprocal(rp[:], dp[:])
                rn = finp.tile([128, H], f32, tag="rn")
                nc.vector.reciprocal(rn[:], dn[:])

                ws_e = finp.tile([128, HD], f32, tag="wse")
                ws_v = _view(ws_[:], ws_[:, 0:H], [[1, H], [0, D]])
                nc.scalar.copy(ws_e[:], ws_v)
                rp_e = finp.tile([128, HD], f32, tag="rpe")
                rp_v = _view(rp[:], rp[:, 0:H], [[1, H], [0, D]])
                nc.scalar.copy(rp_e[:], rp_v)
                rn_e = finp.tile([128, HD], f32, tag="rne")
                rn_v = _view(rn[:], rn[:, 0:H], [[1, H], [0, D]])
                nc.scalar.copy(rn_e[:], rn_v)
                sm_ = finp.tile([128, HD], f32, tag="smv")
                nc.vector.tensor_tensor(sm_[:], zdst_t[w][:], ws_e[:], AT.mult)
                rstp_r = finp.tile([128, HD], f32, tag="rpr")
                nc.vector.tensor_tensor(rstp_r[:], seg_m[:, 0:HD], sm_[:], AT.add)
                rstn_r = finp.tile([128, HD], f32, tag="rnr")
                nc.vector.tensor_tensor(rstn_r[:], rstp_r[:], seg_d[:, 0:HD], AT.subtract)

                rstp = finp.tile([128, HD], f32, tag="rstp")
                nc.vector.tensor_tensor(rstp[:], rstp_r[:], rp_e[:], AT.mult)
                rstn = finp.tile([128, HD], f32, tag="rstn")
                nc.vector.tensor_tensor(rstn[:], rstn_r[:], rn_e[:], AT.mult)

                if has_bias:
                    nc.vector.tensor_tensor(rstp[:], rstp[:], bias_rep[:], AT.add)
                    nc.vector.tensor_tensor(rstn[:], rstn[:], bias_rep[:], AT.add)
                nc.vector.scalar_tensor_tensor(
                    rstp[:], rstp[:], prelu_bc[:, 0:1], rstp[:], AT.mult, AT.max
                )
                nc.vector.scalar_tensor_tensor(
                    rstn[:], rstn[:], prelu_bc[:, 0:1], rstn[:], AT.mult, AT.max
                )

                h8p = finp.tile([128, D], f32, tag="h8p")
                nc.vector.tensor_reduce(
                    h8p[:], _view(rstp[:], rstp[:, 0:HD], [[1, D], [D, H]]),
                    AX.X, AT.add,
                )
                h8n = finp.tile([128, D], f32, tag="h8n")
                nc.vector.tensor_reduce(
                    h8n[:], _view(rstn[:], rstn[:, 0:HD], [[1, D], [D, H]]),
                    AX.X, AT.add,
                )
                nc.scalar.dma_start(hpos[w * 128 : (w + 1) * 128, :], h8p[:])

                scr = finp.tile([128, D], f32, tag="scr")
                nc.vector.tensor_tensor(scr[:], h8p[:], h8n[:], AT.mult)
                nc.vector.tensor_reduce(nm_buf[:, w : w + 1], scr[:], AX.X, AT.add)
                nc.vector.tensor_tensor(scr[:], h8p[:], h8p[:], AT.mult)
                nc.vector.tensor_reduce(na_buf[:, w : w + 1], scr[:], AX.X, AT.add)
                nc.vector.tensor_tensor(scr[:], h8n[:], h8n[:], AT.mult)
                nc.vector.tensor_reduce(nb_buf[:, w : w + 1], scr[:], AX.X, AT.add)

            # skewed software pipeline: gather(w) || compute(w-1) || final(w-2)
            for w in range(NW + 2):
                if w < NW:
                    stage_gather(w)
                if 1 <= w < NW + 1:
                    stage_compute(w - 1)
                if w >= 2:
                    stage_final(w - 2)

            # ---- batched cosine / LSE tail ----
            pr2 = cp.tile([128, NW], f32)
            nc.vector.tensor_tensor(pr2[:], na_buf[:], nb_buf[:], AT.mult)
            nc.vector.tensor_scalar(pr2[:], pr2[:], 1e-30, None, AT.max)
            sq = cp.tile([128, NW], f32)
            nc.scalar.activation(sq[:], pr2[:], AF.Sqrt)
            rr = cp.tile([128, NW], f32)
            nc.vector.reciprocal(rr[:], sq[:])
            cosv = cp.tile([128, NW], f32)
            nc.vector.tensor_tensor(cosv[:], nm_buf[:], rr[:], AT.mult)
            ctr = cp.tile([128, NW], f32)
            nc.scalar.activation(ctr[:], cosv[:], AF.Exp, scale=inv_tem)
            nc.vector.tensor_tensor(ctr[:], ctr[:], mask_sb[:], AT.mult)
            acc = cp.tile([128, 1], f32)
            nc.vector.tensor_reduce(acc[:], ctr[:], AX.X, AT.add)
            with tc.tile_pool(name="fps", bufs=1, space="PSUM") as fps:
                tot_ps = fps.tile([1, 1], f32)
                nc.tensor.matmul(tot_ps[:], acc[:], ones_col[:])
                tot = cp.tile([1, 1], f32)
                nc.vector.tensor_copy(tot[:], tot_ps[:])
                nc.sync.dma_start(scal[:], tot[:])

    library_overlay.lower_extended_insts(nc)
    _split_sync_waits(nc)
    return nc


# ---------------------------------------------------------------------------
# host side
# ---------------------------------------------------------------------------

_GRAPH_CACHE = {}


def prep_inputs(feat, fc_w, attn_l, attn_r, bias, prelu_a, edge_src, edge_dst,
                neg_idx, cfg=CFG):
    NS, ND, E = cfg["N_SRC"], cfg["N_DST"], cfg["E"]
    NC_, H, D, IN = cfg["NCORES"], cfg["H"], cfg["D"], cfg["IN"]
    DPC = ND // NC_
    NW = (DPC + 127) // 128
    NSP = ((NS + 127) // 128) * 128

    feat = np.asarray(feat, np.float32)
    edge_src = np.asarray(edge_src, np.int64)
    edge_dst = np.asarray(edge_dst, np.int64)
    neg_idx = np.asarray(neg_idx, np.int64)

    keep = np.zeros(E, bool)
    keep[neg_idx] = True

    order = np.argsort(edge_dst, kind="stable")
    src_s = edge_src[order]
    dst_s = edge_dst[order]
    drop_s = ~keep[order]

    ld = dst_s - NS
    core = ld // DPC
    lw = (ld % DPC) // 128
    rel = (ld % DPC) % 128

    key = core * NW + lw
    main_counts = np.bincount(key, minlength=NC_ * NW)
    drop_counts = np.bincount(key[drop_s], minlength=NC_ * NW)
    T = max(1, int(-(-main_counts.max() // 128)))
    TD = max(1, int(-(-drop_counts.max() // 128)))

    def build_imgs(sel_src, sel_key, sel_rel, TL):
        o2 = np.argsort(sel_key, kind="stable")
        ss, kk, rr = sel_src[o2], sel_key[o2], sel_rel[o2]
        starts = np.searchsorted(kk, np.arange(NC_ * NW))
        ends = np.searchsorted(kk, np.arange(NC_ * NW) + 1)
        idx_imgs = np.zeros((NC_, 128, NW * TL * 8), np.int16)
        rel_imgs = np.full((NC_, 128, NW * TL), -1.0, np.float32)
        npad = TL * 128
        for c in range(NC_):
            for w in range(NW):
                k = c * NW + w
                s, e = starts[k], ends[k]
                n = e - s
                idx = np.zeros(npad, np.int16)
                idx[:n] = ss[s:e].astype(np.int16)
                rl = np.full(npad, -1.0, np.float32)
                rl[:n] = rr[s:e].astype(np.float32)
                img16 = idx.reshape(TL * 8, 16).T
                idx_imgs[c][:, w * TL * 8 : (w + 1) * TL * 8] = np.tile(img16, (8, 1))
                rel_imgs[c][:, w * TL : (w + 1) * TL] = rl.reshape(TL, 128).T
        return idx_imgs, rel_imgs

    gm_imgs, relm_imgs = build_imgs(src_s, key, rel, T)
    gd_imgs, reld_imgs = build_imgs(src_s[drop_s], key[drop_s], rel[drop_s], TD)
    TT = T + TD

    featT_src = np.zeros((IN, NSP), np.float32)
    featT_src[:, :NS] = feat[:NS].T

    p = np.arange(128)[:, None]
    k = np.arange(NW)[None, :]
    mask = ((k * 128 + p) < DPC).astype(np.float32)

    try:
        import ml_dtypes
        bfdt = ml_dtypes.bfloat16
    except ImportError:
        import jax.numpy as jnp
        bfdt = jnp.bfloat16
    ident_bf = np.eye(128, dtype=np.float32).astype(bfdt)
    ar128 = np.arange(128, dtype=np.float32)

    def build_onehots(relm_c, reld_c):
        # rel images: [128(e), NW*TL]; returns oh/ohT [128, NW*TT*128] bf16
        rm = relm_c.reshape(128, NW, T)
        rd = reld_c.reshape(128, NW, TD)
        rel_all = np.concatenate([rm, rd], axis=2)  # [128, NW, TT]
        oh = (rel_all[:, :, :, None] == ar128[None, None, None, :])
        import ml_dtypes as _md
        ohb = oh.astype(_md.float8_e4m3).reshape(128, NW * TT * 128)
        # ohT[d, w, t, e] = (rel_all[e, w, t] == d)
        ohT = (rel_all.transpose(1, 2, 0)[None, :, :, :] ==
               ar128[:, None, None, None])
        ohTb = ohT.astype(_md.float8_e4m3).reshape(128, NW * TT * 128)
        return ohb, ohTb

    has_bias = bool(np.any(np.asarray(bias)))

    in_maps = []
    for c in range(NC_):
        oh_c, ohT_c = build_onehots(relm_imgs[c], reld_imgs[c])
        fdT = np.zeros((IN, NW * 128), np.float32)
        fdT[:, :DPC] = feat[NS + c * DPC : NS + (c + 1) * DPC].T
        in_maps.append(
            dict(
                featT_src=featT_src,
                featT_dst=fdT,
                fc_w=np.asarray(fc_w, np.float32),
                attn_l=np.asarray(attn_l, np.float32),
                attn_r=np.asarray(attn_r, np.float32),
                bias=np.asarray(bias, np.float32).reshape(1, -1),
                prelu=np.asarray(prelu_a, np.float32).reshape(1, 1),
                gm_idx=gm_imgs[c],
                gd_idx=gd_imgs[c],
                mask=mask,
                ident_bf=ident_bf,
                oh_all=oh_c,
                ohT_all=ohT_c,
            )
        )
    return in_maps, T, TD, has_bias


def run(inputs, trace=False, cfg=CFG):
    in_maps, T, TD, has_bias = prep_inputs(**inputs, cfg=cfg)
    ck = (T, TD, has_bias, id(cfg) if cfg is not CFG else 0)
    if ck not in _GRAPH_CACHE:
        _GRAPH_CACHE[ck] = build_graph(T, TD, has_bias, cfg)
    nc = _GRAPH_CACHE[ck]
    if trace:
        import antenv.axon_hooks as ah
        ah.register_default()
    res = run_bass_kernel_spmd(
        nc, in_maps, core_ids=list(range(cfg["NCORES"])), trace=trace
    )
    NC_, ND = cfg["NCORES"], cfg["N_DST"]
    DPC = ND // NC_
    hp = np.concatenate([res.results[c]["hpos"][:DPC] for c in range(NC_)], axis=0)
    s = np.sum([np.float64(res.results[c]["scal"][0, 0]) for c in range(NC_)])
    loss = np.float32(np.log(s))
    return (np.asarray(loss, np.float32), hp), res


def kernel(**inputs):
    out, _res = run(inputs, trace=False)
    return out
